# revision 24
# baseline (speedup 1.0000x reference)
"""DGCNN forward pass on 8 Trainium2 NeuronCores (Bass/Tile).

Sharding: data-parallel over batch B=8, one point cloud per core. Only BN batch
statistics (5 all-reduces) and the pooled features for the classifier (1
all-gather) cross cores; the classifier is computed redundantly on every core.

Algorithmic notes:
  - Edge conv is linear, so it commutes with the neighbor gather:
        h[:, n, j] = W @ [f_idx - f_n; f_n] = A[:, idx(n,j)] + Bd[:, n]
    with A = Wa @ X and Bd = (Wb - Wa) @ X. A is materialized point-major in
    DRAM and rows are gathered by KNN index via indirect DMA.
  - BN (training stats) + LeakyReLU are monotone per channel, so max over the
    k neighbors commutes with them (for gamma >= 0): we max-reduce the raw
    gathered A rows, add Bd, and apply BN+lrelu once per point.
  - BN statistics over edges decompose:  sum_e h = sum_p S + k*sum_p Bd  and
    sum_e h^2 = sum_p Q + 2*sum_p S.Bd + k*sum_p Bd^2, with per-point
    S = sum_j A_gathered (DVE strided reduce over the gathered tile) and
    Q = sum_j A_gathered^2 (ACT in-place square + DVE reduce). The per-tile
    loop is software-pipelined: top-k of tile t+1 is issued before the
    gather/reduce phase of tile t so DVE and GpSimd overlap.
  - Top-20 neighbor indices per point: ranking key 2*X^T X - ||x_j||^2 (the
    per-row -||x_i||^2 term does not change the order), 3 rounds of the DVE
    max8 / max_index / match_replace instructions.
"""
import sys
import types
import functools

sys.path.insert(0, "/opt/trn_rl_repo")

import numpy as np
import orjson

# ---------------------------------------------------------------------------
# antenv.axon_hooks shim (missing in this container; needed for trace=True)
# ---------------------------------------------------------------------------
if "antenv.axon_hooks" not in sys.modules:
    _m = types.ModuleType("antenv.axon_hooks")
    _m._hook = None

    def _set_hook(h):
        _m._hook = h

    def _get_hook():
        return _m._hook

    _m.set_axon_ntff_profile_hook = _set_hook
    _m.get_axon_ntff_profile_hook = _get_hook
    sys.modules["antenv.axon_hooks"] = _m
    try:
        import antenv

        antenv.axon_hooks = _m
    except ImportError:
        pass

import concourse.bass as bass
import concourse.tile as tile
from concourse import mybir
from concourse import bass_utils
from concourse.masks import make_identity

f32 = mybir.dt.float32
u32 = mybir.dt.uint32
AF = mybir.ActivationFunctionType
OP = mybir.AluOpType
AX = mybir.AxisListType

N = 2048            # points per cloud
K = 20              # neighbors
NB = 8              # batch size == number of cores
T = N // 128        # 16 point tiles per cloud
EPS = 1e-5
ALPHA = 0.2
EDGE_LAYERS = [(3, 64), (64, 64), (64, 128), (128, 256)]  # (C_in, O)
NEG_BIG = -3.0e38

# ---------------------------------------------------------------------------
# BIR patch: this container's walrus accepts only ONE sync-wait per
# instruction; split extra waits onto preceding NoOps on the same engine.
# ---------------------------------------------------------------------------


def _split_multiwaits_json(bir_bytes: bytes) -> bytes:
    bir = orjson.loads(bir_bytes)
    ctr = [0]

    def process_block(block):
        insts = block.get("instructions", [])
        out = []
        changed = False
        for i in insts:
            si = i.get("sync_info")
            waits = (si or {}).get("on_wait") or []
            if len(waits) > 1:
                changed = True
                for w in waits[:-1]:
                    ctr[0] += 1
                    out.append({
                        "engine": i["engine"],
                        "ins": [],
                        "outs": [],
                        "name": f"WS-{ctr[0]}",
                        "opcode": "NoOp",
                        "sync_info": {"on_update": [], "on_wait": [w]},
                    })
                si["on_wait"] = [waits[-1]]
            out.append(i)
        if changed:
            block["instructions"] = out
        for b in block.get("blocks", []) or []:
            process_block(b)

    for f in bir["functions"]:
        for b in f["blocks"]:
            process_block(b)
    return orjson.dumps(bir)


def _patch_bass(nc):
    orig = nc.to_json_bytes
    nc.to_json_bytes = lambda: _split_multiwaits_json(orig())
    return nc


# ---------------------------------------------------------------------------
# Kernel builder
# ---------------------------------------------------------------------------


def _build(dbg=False):
    nc = bass.Bass("TRN2", target_bir_lowering=False, debug=False, num_devices=NB)

    # ---- DRAM I/O ----
    x_in = nc.dram_tensor("x", [3, N], f32, kind="ExternalInput")
    wa, wd, g_in, b_in = [], [], [], []
    for li, (C, O) in enumerate(EDGE_LAYERS):
        wa.append(nc.dram_tensor(f"wa{li}", [C, O], f32, kind="ExternalInput"))
        wd.append(nc.dram_tensor(f"wd{li}", [C, O], f32, kind="ExternalInput"))
        g_in.append(nc.dram_tensor(f"g{li}", [1, O], f32, kind="ExternalInput"))
        b_in.append(nc.dram_tensor(f"b{li}", [1, O], f32, kind="ExternalInput"))
    w5_in = nc.dram_tensor("w5t", [512, 1024], f32, kind="ExternalInput")
    g5_in = nc.dram_tensor("g5c", [128, 8], f32, kind="ExternalInput")  # col-major per O-tile
    b5_in = nc.dram_tensor("b5c", [128, 8], f32, kind="ExternalInput")
    wl1_in = nc.dram_tensor("wl1t", [2048, 512], f32, kind="ExternalInput")
    g6_in = nc.dram_tensor("g6", [1, 512], f32, kind="ExternalInput")
    b6_in = nc.dram_tensor("b6", [1, 512], f32, kind="ExternalInput")
    wl2_in = nc.dram_tensor("wl2t", [512, 256], f32, kind="ExternalInput")
    bl2_in = nc.dram_tensor("bl2", [1, 256], f32, kind="ExternalInput")
    g7_in = nc.dram_tensor("g7", [1, 256], f32, kind="ExternalInput")
    b7_in = nc.dram_tensor("b7", [1, 256], f32, kind="ExternalInput")
    wl3_in = nc.dram_tensor("wl3t", [256, 40], f32, kind="ExternalInput")
    bl3_in = nc.dram_tensor("bl3", [1, 40], f32, kind="ExternalInput")
    out_dram = nc.dram_tensor("out", [NB, 40], f32, kind="ExternalOutput")
    if dbg:
        dbg_x1 = nc.dram_tensor("dbg_x1", [64, N], f32, kind="ExternalOutput")
        dbg_d0 = nc.dram_tensor("dbg_d0", [128, N], f32, kind="ExternalOutput")
        dbg_ii0 = nc.dram_tensor("dbg_ii0", [128, 24], u32, kind="ExternalOutput")
        dbg_g0 = nc.dram_tensor("dbg_g0", [128, 64], f32, kind="ExternalOutput")
        dbg_st1 = nc.dram_tensor("dbg_st1", [1, 128], f32, kind="ExternalOutput")
        dbg_x4 = nc.dram_tensor("dbg_x4", [128, N], f32, kind="ExternalOutput")
        dbg_prow = nc.dram_tensor("dbg_prow", [16, 128], f32, kind="ExternalOutput")
        dbg_pall = nc.dram_tensor("dbg_pall", [8, 2048], f32, kind="ExternalOutput")

    # internal DRAM
    a_tab = [nc.dram_tensor(f"atab{li}", [N, O], f32) for li, (C, O) in enumerate(EDGE_LAYERS)]
    a2_tab = [nc.dram_tensor(f"a2tab{li}", [N, O], f32) for li, (C, O) in enumerate(EDGE_LAYERS)]
    cc_in = [nc.dram_tensor(f"ccin{li}", [1, 2 * O], f32) for li, (C, O) in enumerate(EDGE_LAYERS)]
    cc_out = [nc.dram_tensor(f"ccout{li}", [1, 2 * O], f32, addr_space="Shared")
              for li, (C, O) in enumerate(EDGE_LAYERS)]
    cc5_in = nc.dram_tensor("cc5in", [128, 16], f32)
    cc5_out = nc.dram_tensor("cc5out", [128, 16], f32, addr_space="Shared")
    ag_in = nc.dram_tensor("agin", [16, 128], f32)
    ag_out = nc.dram_tensor("agout", [NB, 16, 128], f32, addr_space="Shared")

    groups = [list(range(NB))]

    with tile.TileContext(nc) as tc:
        import contextlib

        with contextlib.ExitStack() as ctx:
            const = ctx.enter_context(tc.tile_pool(name="const", bufs=1))
            feat = ctx.enter_context(tc.tile_pool(name="feat", bufs=1))

            ident = const.tile([128, 128], f32)
            make_identity(nc, ident)
            ones_row = const.tile([1, N], f32)
            nc.gpsimd.memset(ones_row[:], 1.0)
            ones_col = const.tile([128, 1], f32)
            nc.gpsimd.memset(ones_col[:], 1.0)
            ones8 = const.tile([8, 1], f32)
            nc.gpsimd.memset(ones8[:], 1.0)
            eps_col = const.tile([128, 1], f32)
            nc.gpsimd.memset(eps_col[:], EPS)

            # persistent feature slabs forming `cat` for the W5 matmul
            slab12 = feat.tile([128, N], f32)   # x1 rows 0:64, x2 rows 64:128
            slab3 = feat.tile([128, N], f32)    # x3 (also the L4 input)
            slab4a = feat.tile([128, N], f32)   # x4 channels 0:128
            slab4b = feat.tile([128, N], f32)   # x4 channels 128:256

            # -------------------- edge layers --------------------
            aug_stack = contextlib.ExitStack()
            augp = aug_stack.enter_context(tc.tile_pool(name="augp", bufs=1))
            aug_a = augp.tile([65, N], f32)     # layer input (+ ones row)
            aug_b = augp.tile([65, N], f32)
            nc.sync.dma_start(aug_a[0:3, :], x_in.ap())
            nc.sync.dma_start(aug_a[3:4, :], ones_row[:])
            aug_of = [(aug_a, 4), (aug_b, 65), (aug_a, 65), (slab3, 128)]

            for li, (C, O) in enumerate(EDGE_LAYERS):
                X, _rows = aug_of[li]   # [C+1, N] augmented (L4: slab3, no ones)
                is_l4 = li == 3
                oc_n = (O + 127) // 128

                with contextlib.ExitStack() as lctx:
                    lconst = lctx.enter_context(tc.tile_pool(name=f"lc{li}", bufs=1))
                    lwork = lctx.enter_context(tc.tile_pool(name=f"lw{li}", bufs=1))
                    dwork = lctx.enter_context(tc.tile_pool(name=f"dw{li}", bufs=(3 if is_l4 else 4)))
                    gpool = lctx.enter_context(tc.tile_pool(name=f"gp{li}", bufs=(2 if is_l4 else 3)))
                    spool = lctx.enter_context(tc.tile_pool(name=f"sp{li}", bufs=(2 if is_l4 else 3)))
                    atpool = lctx.enter_context(tc.tile_pool(name=f"ap{li}", bufs=3))
                    ppool = lctx.enter_context(tc.tile_pool(name=f"pp{li}", bufs=1, space="PSUM"))

                    wat = lconst.tile([C, O], f32)
                    nc.sync.dma_start(wat[:], wa[li].ap())
                    wdt = lconst.tile([C, O], f32)
                    nc.sync.dma_start(wdt[:], wd[li].ap())

                    # xx = sum_c X^2 as [1, N]; ranking rhs = [2X; -xx]
                    x2sq = dwork.tile([C, N], f32, tag="d")
                    nc.scalar.activation(x2sq[:], X[0:C, :], AF.Square)
                    if is_l4:
                        negxx = lwork.tile([1, N], f32, name="negxx")
                        rhs2x = lwork.tile([128, N], f32, name="rhs2x")
                        nc.scalar.activation(rhs2x[:], X[0:C, :], AF.Copy, scale=2.0)
                        rhs_aug = None
                    else:
                        rhs_aug = lwork.tile([C + 1, N], f32, name="rhsaug")
                        nc.scalar.activation(rhs_aug[0:C, :], X[0:C, :], AF.Copy, scale=2.0)
                        negxx = lwork.tile([1, N], f32, name="negxxrow")
                    for b in range(4):
                        pxx = ppool.tile([1, 512], f32, tag="small", bufs=1)
                        nc.tensor.matmul(pxx[:], ones_col[0:C, :], x2sq[:, bass.ts(b, 512)],
                                         start=True, stop=True)
                        nc.scalar.activation(negxx[0:1, bass.ts(b, 512)], pxx[:], AF.Copy,
                                             scale=-1.0)
                    if not is_l4:
                        nc.sync.dma_start(rhs_aug[C:C + 1, :], negxx[:])

                    # A/B matmuls; A and A^2 tables to DRAM; B^T kept in SBUF
                    bt_slab = lwork.tile([128, T * O], f32)
                    mx_slab = lwork.tile([128, T * O], f32)
                    for t in range(T):
                        pa = ppool.tile([128, O], f32, tag="pa", bufs=1)
                        nc.tensor.matmul(pa[:], X[0:C, bass.ts(t, 128)], wat[:],
                                         start=True, stop=True)
                        at = atpool.tile([128, O], f32, tag="at")
                        nc.scalar.activation(at[:], pa[:], AF.Copy)
                        a2t = atpool.tile([128, O], f32, tag="a2t")
                        nc.scalar.activation(a2t[:], pa[:], AF.Square)
                        nc.sync.dma_start(a_tab[li].ap()[bass.ts(t, 128), :], at[:])
                        nc.sync.dma_start(a2_tab[li].ap()[bass.ts(t, 128), :], a2t[:])
                        pb = ppool.tile([128, O], f32, tag="pa", bufs=1)
                        nc.tensor.matmul(pb[:], X[0:C, bass.ts(t, 128)], wdt[:],
                                         start=True, stop=True)
                        nc.scalar.activation(bt_slab[:, bass.ts(t, O)], pb[:], AF.Copy)

                    # stats accumulators
                    acc_s = lwork.tile([128, O], f32)
                    nc.vector.memset(acc_s[:], 0.0)
                    acc_q = lwork.tile([128, O], f32)
                    nc.vector.memset(acc_q[:], 0.0)
                    acc_x = lwork.tile([128, O], f32)
                    nc.vector.memset(acc_x[:], 0.0)
                    acc_b = lwork.tile([128, O], f32)
                    acc_b2 = lwork.tile([128, O], f32)

                    def phase_topk(t):
                        # distance ranking tile D = 2 X^T X - xx_j  [128, N]
                        D = dwork.tile([128, N], f32, tag="d")
                        for b in range(4):
                            pd = ppool.tile([128, 512], f32, tag="pd", bufs=4)
                            if is_l4:
                                nc.tensor.matmul(pd[:], X[0:C, bass.ts(t, 128)],
                                                 rhs2x[:, bass.ts(b, 512)],
                                                 start=True, stop=False)
                                nc.tensor.matmul(pd[:], ones_row[:, bass.ts(t, 128)],
                                                 negxx[0:1, bass.ts(b, 512)],
                                                 start=False, stop=True)
                            else:
                                nc.tensor.matmul(pd[:], X[0:C + 1, bass.ts(t, 128)],
                                                 rhs_aug[:, bass.ts(b, 512)],
                                                 start=True, stop=True)
                            nc.scalar.activation(D[:, bass.ts(b, 512)], pd[:], AF.Copy)

                        # top-20 (of 24) indices via 3 rounds of max8
                        mv = spool.tile([128, 24], f32, tag="mv")
                        ii = spool.tile([128, 24], u32, tag="ii")
                        Dm = dwork.tile([128, N], f32, tag="d")
                        Dm2 = dwork.tile([128, N], f32, tag="d")
                        nc.vector.max(mv[:, 0:8], D[:])
                        nc.vector.max_index(ii[:, 0:8], mv[:, 0:8], D[:])
                        nc.vector.match_replace(Dm[:], mv[:, 0:8], D[:], NEG_BIG)
                        nc.vector.max(mv[:, 8:16], Dm[:])
                        nc.vector.max_index(ii[:, 8:16], mv[:, 8:16], Dm[:])
                        nc.vector.match_replace(Dm2[:], mv[:, 8:16], Dm[:], NEG_BIG)
                        nc.vector.max(mv[:, 16:24], Dm2[:])
                        nc.vector.max_index(ii[:, 16:24], mv[:, 16:24], Dm2[:])
                        return ii

                        if dbg and li == 0 and t == 0:
                            nc.sync.dma_start(dbg_d0.ap(), D[:])
                            nc.sync.dma_start(dbg_ii0.ap(), ii[:])

                    def phase_gather(t, ii):
                        # gathers: full slots (for max), per-point sums from gall
                        gall = gpool.tile([128, K * O], f32, tag="gall")
                        for j in range(K):
                            nc.gpsimd.indirect_dma_start(
                                out=gall[:, bass.ts(j, O)], out_offset=None,
                                in_=a_tab[li].ap(),
                                in_offset=bass.IndirectOffsetOnAxis(ap=ii[:, j:j + 1], axis=0))
                        # neighbor max first, then per-point sums from gall
                        nc.vector.tensor_reduce(
                            mx_slab[:, bass.ts(t, O)],
                            gall[:].rearrange("p (j o) -> p o j", j=K),
                            axis=AX.X, op=OP.max)
                        st = spool.tile([128, O], f32, tag="st")
                        nc.vector.tensor_reduce(
                            st[:], gall[:].rearrange("p (j o) -> p o j", j=K),
                            axis=AX.X, op=OP.add)
                        # square gall in place, then sum for Q
                        nc.scalar.activation(gall[:], gall[:], AF.Square)
                        qt = spool.tile([128, O], f32, tag="qt")
                        nc.vector.tensor_reduce(
                            qt[:], gall[:].rearrange("p (j o) -> p o j", j=K),
                            axis=AX.X, op=OP.add)
                        if dbg and li == 0 and t == 0:
                            nc.sync.dma_start(dbg_g0.ap(), gall[:, 0:64])
                        nc.vector.tensor_add(acc_s[:], acc_s[:], st[:])
                        nc.vector.tensor_add(acc_q[:], acc_q[:], qt[:])
                        tmp = spool.tile([128, O], f32, tag="tx")
                        nc.vector.tensor_mul(tmp[:], st[:], bt_slab[:, bass.ts(t, O)])
                        nc.vector.tensor_add(acc_x[:], acc_x[:], tmp[:])
                        nc.vector.tensor_add(mx_slab[:, bass.ts(t, O)],
                                             mx_slab[:, bass.ts(t, O)],
                                             bt_slab[:, bass.ts(t, O)])

                    prev = None
                    for t in range(T):
                        ii_t = phase_topk(t)
                        if prev is not None:
                            phase_gather(prev[0], prev[1])
                        prev = (t, ii_t)
                    phase_gather(prev[0], prev[1])

                    # ---- layer stats finalize + allreduce ----
                    nc.vector.tensor_reduce(acc_b[:],
                                            bt_slab[:].rearrange("p (t o) -> p o t", t=T),
                                            axis=AX.X, op=OP.add)
                    half = T * O // 2
                    b2h = lwork.tile([128, O], f32, name="b2h")
                    for hh in range(2):
                        btsq = dwork.tile([128, half], f32, tag="d", name=f"btsq{hh}")
                        nc.scalar.activation(btsq[:, 0:half],
                                             bt_slab[:, hh * half:(hh + 1) * half], AF.Square)
                        dst = acc_b2 if hh == 0 else b2h
                        nc.vector.tensor_reduce(
                            dst[:],
                            btsq[:, 0:half].rearrange("p (t o) -> p o t", t=T // 2),
                            axis=AX.X, op=OP.add)
                    nc.vector.tensor_add(acc_b2[:], acc_b2[:], b2h[:])
                    stat = lwork.tile([1, 2 * O], f32)
                    # sum_h = colsum(acc_s + K*acc_b)
                    # sumsq_h = colsum(acc_q + 2*acc_x + K*acc_b2)
                    w_s = lwork.tile([128, O], f32)
                    nc.vector.tensor_scalar(w_s[:], acc_b[:], float(K), None, op0=OP.mult)
                    nc.vector.tensor_add(w_s[:], w_s[:], acc_s[:])
                    w_q = lwork.tile([128, O], f32)
                    nc.vector.tensor_scalar(w_q[:], acc_b2[:], float(K), None, op0=OP.mult)
                    nc.vector.tensor_add(w_q[:], w_q[:], acc_q[:])
                    tmp3 = lwork.tile([128, O], f32)
                    nc.vector.tensor_scalar(tmp3[:], acc_x[:], 2.0, None, op0=OP.mult)
                    nc.vector.tensor_add(w_q[:], w_q[:], tmp3[:])
                    for (src_t, off) in ((w_s, 0), (w_q, O)):
                        pstat = ppool.tile([1, O], f32, tag="small", bufs=1, name=f"ps{off}")
                        nc.tensor.matmul(pstat[:], ones_col[:], src_t[:], start=True, stop=True)
                        nc.scalar.activation(stat[0:1, off:off + O], pstat[:], AF.Copy)

                    nc.sync.dma_start(cc_in[li].ap(), stat[:])
                    nc.gpsimd.collective_compute(
                        "AllReduce", OP.add, replica_groups=groups,
                        ins=[cc_in[li].ap()], outs=[cc_out[li].ap()])
                    stat_r = lwork.tile([1, 2 * O], f32)
                    nc.sync.dma_start(stat_r[:], cc_out[li].ap())
                    if dbg and li == 0:
                        nc.sync.dma_start(dbg_st1.ap(), stat_r[:])

                    # mu, var, scale, bias
                    inv_cnt = 1.0 / (NB * N * K)
                    mu = lwork.tile([1, O], f32)
                    nc.vector.tensor_scalar(mu[:], stat_r[0:1, 0:O], inv_cnt, None, op0=OP.mult)
                    var = lwork.tile([1, O], f32)
                    nc.vector.tensor_scalar(var[:], stat_r[0:1, O:2 * O], inv_cnt, None,
                                            op0=OP.mult)
                    musq = lwork.tile([1, O], f32)
                    nc.vector.tensor_mul(musq[:], mu[:], mu[:])
                    nc.vector.tensor_sub(var[:], var[:], musq[:])
                    sd = lwork.tile([1, O], f32)
                    nc.scalar.activation(sd[:], var[:], AF.Sqrt, bias=eps_col[0:1, :])
                    rs = lwork.tile([1, O], f32)
                    nc.vector.reciprocal(rs[:], sd[:])
                    grow = lwork.tile([1, O], f32)
                    nc.sync.dma_start(grow[:], g_in[li].ap())
                    brow = lwork.tile([1, O], f32)
                    nc.sync.dma_start(brow[:], b_in[li].ap())
                    s_row = lwork.tile([1, O], f32)
                    nc.vector.tensor_mul(s_row[:], rs[:], grow[:])
                    be_row = lwork.tile([1, O], f32)
                    nc.vector.tensor_mul(be_row[:], mu[:], s_row[:])
                    nc.vector.tensor_sub(be_row[:], brow[:], be_row[:])

                    # transpose scale/bias to per-partition columns
                    s_col = lwork.tile([128, oc_n], f32)
                    be_col = lwork.tile([128, oc_n], f32)
                    for oc in range(oc_n):
                        ow = min(128, O - oc * 128)
                        pt1 = ppool.tile([128, 1], f32, tag="small", bufs=1, name=f"pt1{oc}")
                        nc.tensor.matmul(pt1[0:ow, :], s_row[0:1, oc * 128:oc * 128 + ow],
                                         ident[0:1, 0:1], is_transpose=True)
                        nc.scalar.activation(s_col[0:ow, oc:oc + 1], pt1[0:ow, :], AF.Copy)
                        pt2 = ppool.tile([128, 1], f32, tag="small", bufs=1, name=f"pt2{oc}")
                        nc.tensor.matmul(pt2[0:ow, :], be_row[0:1, oc * 128:oc * 128 + ow],
                                         ident[0:1, 0:1], is_transpose=True)
                        nc.scalar.activation(be_col[0:ow, oc:oc + 1], pt2[0:ow, :], AF.Copy)

                    # ---- outputs: x_out = Prelu(s*(Mx + B) + be), transposed ----
                    if li == 0:
                        dests = [aug_b]
                    elif li == 1:
                        dests = [aug_a]
                    elif li == 2:
                        dests = [slab3]
                    else:
                        dests = [slab4a, slab4b]

                    for t in range(T):
                        hmax = mx_slab[:, bass.ts(t, O)]
                        for oc in range(oc_n):
                            ow = min(128, O - oc * 128)
                            ptr = ppool.tile([128, 128], f32, tag="ptr", bufs=2)
                            nc.tensor.matmul(ptr[0:ow, :], hmax[:, oc * 128:oc * 128 + ow],
                                             ident[:], is_transpose=True)
                            dest = dests[oc] if is_l4 else dests[0]
                            nc.scalar.activation(
                                dest[0:ow, bass.ts(t, 128)], ptr[0:ow, :], AF.Prelu,
                                bias=be_col[0:ow, oc:oc + 1],
                                scale=s_col[0:ow, oc:oc + 1], alpha=ALPHA)

                    # ones row for the next layer's augmented input; slab copies
                    if li == 0:
                        nc.sync.dma_start(aug_b[64:65, :], ones_row[:])
                        nc.sync.dma_start(slab12[0:64, :], aug_b[0:64, :])
                    elif li == 1:
                        nc.sync.dma_start(aug_a[64:65, :], ones_row[:])
                        nc.sync.dma_start(slab12[64:128, :], aug_a[0:64, :])

                if li == 0 and dbg:
                    nc.sync.dma_start(dbg_x1.ap(), aug_b[0:64, :])
                if li == 3 and dbg:
                    nc.sync.dma_start(dbg_x4.ap(), slab4a[:])
                if li == 2:
                    aug_stack.close()

            # -------------------- W5 conv + global pooling --------------------
            with contextlib.ExitStack() as wctx:
                wconst = wctx.enter_context(tc.tile_pool(name="w5c", bufs=1))
                hpool = wctx.enter_context(tc.tile_pool(name="hp", bufs=1))
                wwork = wctx.enter_context(tc.tile_pool(name="ww", bufs=2))
                pw = wctx.enter_context(tc.tile_pool(name="pw", bufs=1, space="PSUM"))

                f32r = mybir.dt.float32r
                w5t = [wconst.tile([128, 1024], f32r, name=f"w5t{kt}") for kt in range(4)]
                w5tmp = wconst.tile([128, 1024], f32, name="w5tmp")
                for kt in range(4):
                    nc.sync.dma_start(w5tmp[:], w5_in.ap()[bass.ts(kt, 128), :])
                    nc.vector.tensor_copy(w5t[kt][:], w5tmp[:])
                slabs_f32 = [slab12, slab3, slab4a, slab4b]
                slabs = [wconst.tile([128, N], f32r, name=f"slr{i}") for i in range(4)]
                for i in range(4):
                    nc.vector.tensor_copy(slabs[i][:], slabs_f32[i][:])

                h_sb = [hpool.tile([128, N], f32, name=f"hsb{ot}") for ot in range(8)]
                stat5 = wwork.tile([128, 16], f32)
                hs_parts = wwork.tile([128, 4], f32, tag="hsp")
                hq_parts = wwork.tile([128, 4], f32, tag="hqp")
                for ot in range(8):
                    for b in range(4):
                        ph = pw.tile([128, 512], f32, tag="ph", bufs=4)
                        for kt in range(4):
                            nc.tensor.matmul(ph[:], w5t[kt][:, bass.ts(ot, 128)],
                                             slabs[kt][:, bass.ts(b, 512)],
                                             start=(kt == 0), stop=(kt == 3))
                        nc.scalar.activation(h_sb[ot][:, bass.ts(b, 512)], ph[:], AF.Copy,
                                             accum_out=hs_parts[:, b:b + 1])
                        scr = wwork.tile([128, 512], f32, tag="scr")
                        nc.scalar.activation(scr[:], ph[:], AF.Square,
                                             accum_out=hq_parts[:, b:b + 1])
                    nc.vector.tensor_reduce(stat5[:, ot:ot + 1], hs_parts[:],
                                            axis=AX.X, op=OP.add)
                    nc.vector.tensor_reduce(stat5[:, 8 + ot:9 + ot], hq_parts[:],
                                            axis=AX.X, op=OP.add)

                nc.sync.dma_start(cc5_in.ap(), stat5[:])
                nc.gpsimd.collective_compute(
                    "AllReduce", OP.add, replica_groups=groups,
                    ins=[cc5_in.ap()], outs=[cc5_out.ap()])
                stat5r = wwork.tile([128, 16], f32)
                nc.sync.dma_start(stat5r[:], cc5_out.ap())

                inv5 = 1.0 / (NB * N)
                mu5 = wwork.tile([128, 8], f32)
                nc.vector.tensor_scalar(mu5[:], stat5r[:, 0:8], inv5, None, op0=OP.mult)
                var5 = wwork.tile([128, 8], f32)
                nc.vector.tensor_scalar(var5[:], stat5r[:, 8:16], inv5, None, op0=OP.mult)
                mu5sq = wwork.tile([128, 8], f32)
                nc.vector.tensor_mul(mu5sq[:], mu5[:], mu5[:])
                nc.vector.tensor_sub(var5[:], var5[:], mu5sq[:])
                sd5 = wwork.tile([128, 8], f32)
                nc.scalar.activation(sd5[:], var5[:], AF.Sqrt, bias=eps_col[:, :])
                rs5 = wwork.tile([128, 8], f32)
                nc.vector.reciprocal(rs5[:], sd5[:])
                g5c = wwork.tile([128, 8], f32)
                nc.sync.dma_start(g5c[:], g5_in.ap())
                b5c = wwork.tile([128, 8], f32)
                nc.sync.dma_start(b5c[:], b5_in.ap())
                s5 = wwork.tile([128, 8], f32)
                nc.vector.tensor_mul(s5[:], rs5[:], g5c[:])
                be5 = wwork.tile([128, 8], f32)
                nc.vector.tensor_mul(be5[:], mu5[:], s5[:])
                nc.vector.tensor_sub(be5[:], b5c[:], be5[:])

                pooled = wwork.tile([128, 16], f32)
                for ot in range(8):
                    hact = wwork.tile([128, N], f32, tag="hact")
                    nc.scalar.activation(hact[:], h_sb[ot][:], AF.Prelu,
                                         bias=be5[:, ot:ot + 1], scale=s5[:, ot:ot + 1],
                                         alpha=ALPHA,
                                         accum_out=pooled[:, 8 + ot:9 + ot])
                    nc.vector.tensor_reduce(pooled[:, ot:ot + 1], hact[:],
                                            axis=AX.X, op=OP.max)
                nc.vector.tensor_scalar(pooled[:, 8:16], pooled[:, 8:16], 1.0 / N, None,
                                        op0=OP.mult)

                # pooled [128, 16] -> [16, 128] -> allgather to [8, 2048]
                ptp = pw.tile([16, 128], f32, tag="ptp", bufs=1)
                nc.tensor.matmul(ptp[:], pooled[:], ident[:], is_transpose=True)
                prow = wwork.tile([16, 128], f32)
                nc.scalar.activation(prow[:], ptp[:], AF.Copy)
                nc.sync.dma_start(ag_in.ap(), prow[:])
                if dbg:
                    nc.sync.dma_start(dbg_prow.ap(), prow[:])
                nc.gpsimd.collective_compute(
                    "AllGather", OP.bypass, replica_groups=groups,
                    ins=[ag_in.ap()], outs=[ag_out.ap()])

            # -------------------- classifier (redundant on all cores) --------
            with contextlib.ExitStack() as cctx:
                wwork = cctx.enter_context(tc.tile_pool(name="cw", bufs=1))
                pw = cctx.enter_context(tc.tile_pool(name="pc", bufs=1, space="PSUM"))

                pall = wwork.tile([8, 2048], f32)
                nc.sync.dma_start(pall[:], ag_out.ap().rearrange("c s p -> c (s p)"))
                if dbg:
                    nc.sync.dma_start(dbg_pall.ap(), pall[:])

                def transpose_rows(src, width, nm):
                    """src [8, width] -> [128, 8*width/128] of K-tiles."""
                    kt_n = width // 128
                    dst = wwork.tile([128, 8 * kt_n], f32, name=f"tr{nm}")
                    for kt in range(kt_n):
                        ptk = pw.tile([128, 8], f32, tag="small", bufs=2, name=f"ptk{nm}{kt}")
                        nc.tensor.matmul(ptk[:], src[0:8, bass.ts(kt, 128)], ident[0:8, 0:8],
                                         is_transpose=True)
                        nc.scalar.activation(dst[:, bass.ts(kt, 8)], ptk[:], AF.Copy)
                    return dst

                def bcast8(row, width, nm):
                    """Materialize a [1, width] row broadcast to [8, width]."""
                    pbc = pw.tile([8, width], f32, tag="pfx", bufs=2, name=f"pbc{nm}")
                    nc.tensor.matmul(pbc[:], ones_row[0:1, 0:8], row[0:1, :],
                                     start=True, stop=True)
                    dst = wwork.tile([8, width], f32, name=f"bc{nm}")
                    nc.scalar.activation(dst[:], pbc[:], AF.Copy)
                    return dst

                def bn_lrelu_rows(h, width, g_d, b_d, nm, extra_bias=None):
                    """BatchNorm over the 8 rows + leaky relu; channels on free."""
                    if extra_bias is not None:
                        eb = wwork.tile([1, width], f32, name=f"eb{nm}")
                        nc.sync.dma_start(eb[:], extra_bias.ap())
                        nc.vector.tensor_add(h[:], h[:], bcast8(eb, width, "eb" + nm)[:])
                    ps = pw.tile([1, width], f32, tag="small", bufs=2, name=f"ps{nm}")
                    nc.tensor.matmul(ps[:], ones8[:], h[:], start=True, stop=True)
                    srow = wwork.tile([1, width], f32, name=f"sr{nm}")
                    nc.scalar.activation(srow[:], ps[:], AF.Copy, scale=1.0 / 8)
                    hsq = wwork.tile([8, width], f32, name=f"hsq{nm}")
                    nc.scalar.activation(hsq[:], h[:], AF.Square)
                    pq = pw.tile([1, width], f32, tag="small", bufs=2, name=f"pq{nm}")
                    nc.tensor.matmul(pq[:], ones8[:], hsq[:], start=True, stop=True)
                    qrow = wwork.tile([1, width], f32, name=f"qr{nm}")
                    nc.scalar.activation(qrow[:], pq[:], AF.Copy, scale=1.0 / 8)
                    musq_ = wwork.tile([1, width], f32, name=f"ms{nm}")
                    nc.vector.tensor_mul(musq_[:], srow[:], srow[:])
                    nc.vector.tensor_sub(qrow[:], qrow[:], musq_[:])
                    sdr = wwork.tile([1, width], f32, name=f"sd{nm}")
                    nc.scalar.activation(sdr[:], qrow[:], AF.Sqrt, bias=eps_col[0:1, :])
                    rsr = wwork.tile([1, width], f32, name=f"rs{nm}")
                    nc.vector.reciprocal(rsr[:], sdr[:])
                    gr = wwork.tile([1, width], f32, name=f"g{nm}")
                    nc.sync.dma_start(gr[:], g_d.ap())
                    br = wwork.tile([1, width], f32, name=f"b{nm}")
                    nc.sync.dma_start(br[:], b_d.ap())
                    sc = wwork.tile([1, width], f32, name=f"sc{nm}")
                    nc.vector.tensor_mul(sc[:], rsr[:], gr[:])
                    bec = wwork.tile([1, width], f32, name=f"be{nm}")
                    nc.vector.tensor_mul(bec[:], srow[:], sc[:])
                    nc.vector.tensor_sub(bec[:], br[:], bec[:])
                    y = wwork.tile([8, width], f32, name=f"y{nm}")
                    nc.vector.tensor_mul(y[:], h[:], bcast8(sc, width, "sc" + nm)[:])
                    nc.vector.tensor_add(y[:], y[:], bcast8(bec, width, "bc" + nm)[:])
                    pos = wwork.tile([8, width], f32, name=f"po{nm}")
                    nc.vector.tensor_scalar(pos[:], y[:], 0.0, None, op0=OP.max)
                    nc.vector.tensor_scalar(y[:], y[:], 0.0, ALPHA, op0=OP.min, op1=OP.mult)
                    nc.vector.tensor_add(y[:], y[:], pos[:])
                    return y

                def fc(src_rows, in_w, out_w, w_dram, nm):
                    kt_n = in_w // 128
                    wsb = wwork.tile([128, kt_n * out_w], f32, name=f"w{nm}")
                    for kt in range(kt_n):
                        nc.sync.dma_start(wsb[:, bass.ts(kt, out_w)],
                                          w_dram.ap()[bass.ts(kt, 128), :])
                    tr = transpose_rows(src_rows, in_w, nm)
                    pf = pw.tile([8, out_w], f32, tag="pfx", bufs=2, name=f"pf{nm}")
                    for kt in range(kt_n):
                        nc.tensor.matmul(pf[:], tr[:, bass.ts(kt, 8)],
                                         wsb[:, bass.ts(kt, out_w)],
                                         start=(kt == 0), stop=(kt == kt_n - 1))
                    dst = wwork.tile([8, out_w], f32, name=f"fc{nm}")
                    nc.scalar.activation(dst[:], pf[:], AF.Copy)
                    return dst

                h1 = fc(pall, 2048, 512, wl1_in, "a")
                h1n = bn_lrelu_rows(h1, 512, g6_in, b6_in, "a")
                h2 = fc(h1n, 512, 256, wl2_in, "b")
                h2n = bn_lrelu_rows(h2, 256, g7_in, b7_in, "b", extra_bias=bl2_in)
                h3 = fc(h2n, 256, 40, wl3_in, "c")
                bl3r = wwork.tile([1, 40], f32)
                nc.sync.dma_start(bl3r[:], bl3_in.ap())
                nc.vector.tensor_add(h3[:], h3[:], bcast8(bl3r, 40, "l3")[:])
                nc.sync.dma_start(out_dram.ap(), h3[:])

    _patch_bass(nc)
    return nc


@functools.lru_cache(maxsize=2)
def _built(dbg=False):
    return _build(dbg)


def _prep_inputs(inputs):
    """Host-side weight prep. Returns the per-core-invariant input map."""
    gp = {}
    ws = [inputs["W1"], inputs["W2"], inputs["W3"], inputs["W4"]]
    gs = [inputs["g1"], inputs["g2"], inputs["g3"], inputs["g4"]]
    bs = [inputs["b1"], inputs["b2"], inputs["b3"], inputs["b4"]]
    for li, (C, O) in enumerate(EDGE_LAYERS):
        W = np.asarray(ws[li], np.float32)
        assert W.shape == (O, 2 * C), (W.shape, (O, 2 * C))
        gp[f"wa{li}"] = np.ascontiguousarray(W[:, :C].T)
        gp[f"wd{li}"] = np.ascontiguousarray((W[:, C:] - W[:, :C]).T)
        g = np.asarray(gs[li], np.float32)
        if (g < 0).any():
            raise NotImplementedError("negative BN gamma needs the min-path")
        gp[f"g{li}"] = g.reshape(1, O)
        gp[f"b{li}"] = np.asarray(bs[li], np.float32).reshape(1, O)
    gp["w5t"] = np.ascontiguousarray(np.asarray(inputs["W5"], np.float32).T)
    g5 = np.asarray(inputs["g5"], np.float32)
    if (g5 < 0).any():
        raise NotImplementedError("negative BN gamma needs the min-path")
    gp["g5c"] = np.ascontiguousarray(g5.reshape(8, 128).T)
    gp["b5c"] = np.ascontiguousarray(np.asarray(inputs["b5"], np.float32).reshape(8, 128).T)
    gp["wl1t"] = np.ascontiguousarray(np.asarray(inputs["Wl1"], np.float32).T)
    gp["g6"] = np.asarray(inputs["g6"], np.float32).reshape(1, 512)
    gp["b6"] = np.asarray(inputs["b6"], np.float32).reshape(1, 512)
    gp["wl2t"] = np.ascontiguousarray(np.asarray(inputs["Wl2"], np.float32).T)
    gp["bl2"] = np.asarray(inputs["bl2"], np.float32).reshape(1, 256)
    gp["g7"] = np.asarray(inputs["g7"], np.float32).reshape(1, 256)
    gp["b7"] = np.asarray(inputs["b7"], np.float32).reshape(1, 256)
    gp["wl3t"] = np.ascontiguousarray(np.asarray(inputs["Wl3"], np.float32).T)
    gp["bl3"] = np.asarray(inputs["bl3"], np.float32).reshape(1, 40)
    return gp


def _run(inputs, trace=False, dbg=False):
    nc = _built(dbg)
    gp = _prep_inputs(inputs)
    x = np.asarray(inputs["x"], np.float32)
    in_maps = [{"x": np.ascontiguousarray(x[c]), **gp} for c in range(NB)]
    kw = {}
    if trace:
        from trn_agent_boot.trn_boot import _ntff_profile_via_ctypes
        from antenv.axon_hooks import set_axon_ntff_profile_hook
        set_axon_ntff_profile_hook(_ntff_profile_via_ctypes("/opt/axon/libaxon_pjrt.so"))
        kw["trace"] = True
    return bass_utils.run_bass_kernel_spmd(nc, in_maps, core_ids=list(range(NB)), **kw)


def kernel(**inputs) -> np.ndarray:
    res = _run(inputs, trace=False)
    return np.asarray(res.results[0]["out"], np.float32).copy()
